# revision 1
# baseline (speedup 1.0000x reference)
"""Causal self-attention (B=4, T=2048, C=1024, H=16) on 8 TRN2 NeuronCores.

Sharding: hybrid batch x head tensor-parallel. Core c handles batch b = c//2
and heads [8*(c%2) : 8*(c%2)+8]. Each core computes QKV for its 8 heads over
its batch, full causal attention for those heads, and a *partial* c_proj
(contribution of its 8 heads to all 2048 tokens of its batch). The host
unshards by summing the two partial outputs of each batch pair; b_proj is
added on-device by the even core of each pair.

Single-pass pipelined structure (per core): one loop over the 4 q-windows of
512 tokens. Per window w: QK projections for that token window (Q kept only
for the window, K appended to a persistent K_T), V for the window's 4 token
blocks, then causal attention for all 8 heads over k-blocks 0..4w+3 (scores
matmul -> exp on Act -> triangular mask on DVE -> attn@V accumulate), per-head
softmax normalization (denominator rides in PSUM row 64 via a ones column in
the V tiles), then the window's partial c_proj with the bias folded in as a
rank-1 accumulate matmul and gpsimd moving PSUM->SBUF for the output DMA.
x is fed transposed (xt [C, T]) and loaded once per window; weights stay
resident in SBUF. All matmuls are float32r (full-rate fp32, moving dim kept
>= 256 everywhere: the 128-wide diagonal chunks are widened to 256 with a
zero-extended triangular mask).
"""

import numpy as np

import concourse.bass as bass
import concourse.mybir as mybir
import concourse.tile as tile
from concourse import bacc
from concourse.bass_utils import run_bass_kernel_spmd

B, T, C = 4, 2048, 1024
H = 16          # total heads
HL = 8          # heads per core
D = 64          # head dim
P = 128
W = 512         # q-window / matmul moving-dim size
NW = T // W     # 4 q windows
KB = T // P     # 16 k blocks
NCHUNK = C // P  # 8 contraction chunks over C
PAIRS = HL // 2  # 4 head-pairs (2 heads per 128-partition tile)
F32 = mybir.dt.float32
F32R = mybir.dt.float32r
EXP = mybir.ActivationFunctionType.Exp
N_CORES = 8
LAG = 2          # scores->attn@V software pipeline depth per head

_CACHE = {}
LAST_RESULTS = None


def build_nc():
    if "nc" in _CACHE:
        return _CACHE["nc"]
    nc = bacc.Bacc(
        "TRN2", target_bir_lowering=False, debug=False, num_devices=N_CORES
    )

    xt = nc.dram_tensor("xt", [C, T], F32R, kind="ExternalInput")
    wqk = nc.dram_tensor("wqk", [C, C], F32R, kind="ExternalInput")
    wv = nc.dram_tensor("wv", [C, HL * D], F32R, kind="ExternalInput")
    bqk = nc.dram_tensor("bqk", [P, 2 * PAIRS], F32, kind="ExternalInput")
    bv = nc.dram_tensor("bv", [P, HL * D], F32, kind="ExternalInput")
    wp = nc.dram_tensor("wp", [HL * D, C], F32R, kind="ExternalInput")
    bpr = nc.dram_tensor("bpr", [P, C], F32, kind="ExternalInput")
    trimask = nc.dram_tensor("trimask", [P, P], F32, kind="ExternalInput")
    trimask2 = nc.dram_tensor("trimask2", [P, 2 * P], F32, kind="ExternalInput")
    onesd = nc.dram_tensor("onesd", [P, P], F32R, kind="ExternalInput")
    out = nc.dram_tensor("out", [T, C], F32, kind="ExternalOutput")

    xt_r = xt[:].rearrange("(a p) t -> p a t", p=P)

    with tile.TileContext(nc) as tc, nc.allow_low_precision(
        reason="float32r tiles for full-rate fp32 PE matmuls"
    ):
        with (
            tc.tile_pool(name="consts", bufs=1) as consts,
            tc.tile_pool(name="waqk", bufs=NCHUNK) as waqk_pool,
            tc.tile_pool(name="wav", bufs=NCHUNK) as wav_pool,
            tc.tile_pool(name="xtw", bufs=1) as xtw_pool,
            tc.tile_pool(name="kt", bufs=1) as kt_pool,
            tc.tile_pool(name="qt", bufs=PAIRS) as qt_pool,
            tc.tile_pool(name="vsb", bufs=1) as v_pool,
            tc.tile_pool(name="attn", bufs=LAG + 1) as attn_pool,
            tc.tile_pool(name="yt", bufs=1) as yt_pool,
            tc.tile_pool(name="wp_sb", bufs=1) as wp_pool,
            tc.tile_pool(name="osb", bufs=2) as o_pool,
            tc.tile_pool(name="norm", bufs=1) as norm_pool,
            tc.tile_pool(name="psum", space="PSUM", bufs=3) as psum,
        ):
            # ---- const tiles
            bqk_t = consts.tile([P, 2 * PAIRS], F32)
            bv_t = consts.tile([P, HL * D], F32)
            tri_t = consts.tile([P, P], F32)
            tri2_t = consts.tile([P, 2 * P], F32)
            ones_row = consts.tile([1, P], F32R)
            bpr_t = consts.tile([P, C], F32)

            waqk_sb = [
                waqk_pool.tile([P, C], F32R, tag="waqk", name=f"waqk{a}")
                for a in range(NCHUNK)
            ]
            wav_sb = [
                wav_pool.tile([P, HL * D], F32R, tag="wav", name=f"wav{a}")
                for a in range(NCHUNK)
            ]
            kt_sb = [
                kt_pool.tile([P, T], F32R, tag=f"kt{pr}", name=f"kt{pr}")
                for pr in range(PAIRS)
            ]
            wp_sb = [
                wp_pool.tile([P, C], F32R, tag=f"wp{ch}", name=f"wp{ch}")
                for ch in range(PAIRS)
            ]
            # V laid out [tok, d] per (head, kblock) as [P, 65] slices
            # (col 64 stays 1.0 so attn@V accumulates softmax denominators).
            v_sb = v_pool.tile([P, HL * KB * 65], F32R)
            v_view = v_sb[:].rearrange("p (h k c) -> p h k c", h=HL, k=KB)
            bv_view = bv_t[:].rearrange("p (h d) -> p h d", h=HL)

            # ---- DMA kickoff, window-0 critical path first: Q-half weight
            # chunks interleaved with xt window-0 chunks, then K halves,
            # then wv; everything else after.
            xtw_tiles = {}

            def xtw_get(w):
                if w not in xtw_tiles:
                    t = xtw_pool.tile([P, NCHUNK * W], F32R, tag="xtw",
                                      name=f"xtw{w}")
                    tv = t[:].rearrange("p (a t) -> p a t", a=NCHUNK)
                    for a in range(NCHUNK):
                        nc.sync.dma_start(
                            tv[:, a, :], xt_r[:, a, w * W : (w + 1) * W]
                        )
                    xtw_tiles[w] = t
                return xtw_tiles[w]

            for a in range(NCHUNK):
                nc.sync.dma_start(
                    waqk_sb[a][:, 0:W], wqk[a * P : (a + 1) * P, 0:W]
                )
                if a == 0:
                    nc.sync.dma_start(bqk_t[:], bqk[:])
                    xtw_get(0)
            for a in range(NCHUNK):
                nc.sync.dma_start(
                    waqk_sb[a][:, W:C], wqk[a * P : (a + 1) * P, W:C]
                )
            for a in range(NCHUNK):
                nc.sync.dma_start(wav_sb[a][:], wv[a * P : (a + 1) * P, :])
            nc.sync.dma_start(bv_t[:], bv[:])
            nc.sync.dma_start(
                v_sb[:].rearrange("p (t c) -> p t c", c=65)[:, :, 64:65],
                onesd[:].rearrange("p (t c) -> p t c", c=1),
            )
            nc.sync.dma_start(ones_row[:], onesd[0:1, :])
            nc.sync.dma_start(tri_t[:], trimask[:])
            nc.sync.dma_start(tri2_t[:], trimask2[:])
            for ch in range(PAIRS):
                nc.sync.dma_start(wp_sb[ch][:], wp[ch * P : (ch + 1) * P, :])
            nc.sync.dma_start(bpr_t[:], bpr[:])

            qt_sb = [None] * PAIRS

            def emit_qk_copy(j, qk_ps, w):
                # move PSUM -> SBUF with the per-qk-column bias added
                if j < PAIRS:
                    qt_sb[j] = qt_pool.tile(
                        [P, W], F32R, tag=f"qt{j}", bufs=1, name=f"qt{j}_{w}"
                    )
                    dest = qt_sb[j][:]
                else:
                    dest = kt_sb[j - PAIRS][:, w * W : (w + 1) * W]
                nc.vector.tensor_scalar(
                    out=dest,
                    in0=qk_ps[:],
                    scalar1=bqk_t[:, j : j + 1],
                    scalar2=None,
                    op0=mybir.AluOpType.add,
                )

            def emit_v_add(i, v_ps, w):
                tb = 4 * w + i
                nc.vector.tensor_add(
                    v_view[:, :, tb, 0:D],
                    v_ps[:].rearrange("p (h d) -> p h d", h=HL),
                    bv_view[:, :, :],
                )

            def emit_qkv_window0():
                xtw = xtw_get(0)
                # chunk-major over 2-tile groups (fl banks) so PE can trail
                # the DMA stream chunk by chunk.
                for jg in range(4):
                    js = (jg, 4 + jg)
                    qk_ps = [
                        psum.tile([P, W], F32, tag="fl", bufs=2,
                                  name=f"qk0_{j}")
                        for j in js
                    ]
                    for a in range(NCHUNK):
                        for t, j in enumerate(js):
                            nc.tensor.matmul(
                                qk_ps[t][:],
                                waqk_sb[a][:, j * P : (j + 1) * P],
                                xtw[:, a * W : (a + 1) * W],
                                start=(a == 0),
                                stop=(a == NCHUNK - 1),
                            )
                    for t, j in enumerate(js):
                        emit_qk_copy(j, qk_ps[t], 0)
                for ig in range(2):
                    iis = (2 * ig, 2 * ig + 1)
                    v_ps = [
                        psum.tile([P, W], F32, tag="fl", bufs=2,
                                  name=f"v0_{i}")
                        for i in iis
                    ]
                    for a in range(NCHUNK):
                        for t, i in enumerate(iis):
                            nc.tensor.matmul(
                                v_ps[t][:],
                                xtw[:, a * W + i * P : a * W + (i + 1) * P],
                                wav_sb[a][:],
                                start=(a == 0),
                                stop=(a == NCHUNK - 1),
                            )
                    for t, i in enumerate(iis):
                        emit_v_add(i, v_ps[t], 0)

            def qkv_window_units(w):
                # windows >= 1: inputs already resident, j-major streaming.
                # Returns one closure per projection unit so the caller can
                # interleave them between attention heads as PE filler.
                xtw = xtw_get(w)

                def qk_unit(j):
                    def emit():
                        qk_ps = psum.tile([P, W], F32, tag="fl", bufs=2,
                                          name=f"qk{w}_{j}")
                        for a in range(NCHUNK):
                            nc.tensor.matmul(
                                qk_ps[:],
                                waqk_sb[a][:, j * P : (j + 1) * P],
                                xtw[:, a * W : (a + 1) * W],
                                start=(a == 0),
                                stop=(a == NCHUNK - 1),
                            )
                        emit_qk_copy(j, qk_ps, w)
                    return emit

                def v_unit(i):
                    def emit():
                        v_ps = psum.tile([P, W], F32, tag="fl", bufs=2,
                                         name=f"v{w}_{i}")
                        for a in range(NCHUNK):
                            nc.tensor.matmul(
                                v_ps[:],
                                xtw[:, a * W + i * P : a * W + (i + 1) * P],
                                wav_sb[a][:],
                                start=(a == 0),
                                stop=(a == NCHUNK - 1),
                            )
                        emit_v_add(i, v_ps, w)
                    return emit

                # per-head filler schedule: pair p's Q tile (bufs=1) is
                # only dead after head 2p+1 of the current window, so its
                # qk units may not be emitted earlier; V slots are disjoint.
                return {
                    0: [],
                    1: [qk_unit(0), qk_unit(4)],
                    2: [v_unit(0)],
                    3: [qk_unit(1), qk_unit(5)],
                    4: [v_unit(1)],
                    5: [qk_unit(2), qk_unit(6)],
                    6: [v_unit(2)],
                    7: [qk_unit(3), qk_unit(7), v_unit(3)],
                    "flat": [qk_unit(j) for j in range(2 * PAIRS)]
                    + [v_unit(i) for i in range(4)],
                }

            emit_qkv_window0()

            def cproj_units(w, yt_tiles):
                # partial c_proj of a finished window's 4 token blocks, one
                # closure per (tb, ew) group so they can interleave as PE
                # filler inside the next window's attention.
                def unit(i, ew):
                    def emit():
                        tb = 4 * w + i
                        o_ps = psum.tile([P, W], F32, tag="fl", bufs=2,
                                         name=f"o{tb}_{ew}")
                        for ch in range(PAIRS):
                            nc.tensor.matmul(
                                o_ps[:],
                                yt_tiles[ch][:, i * P : (i + 1) * P],
                                wp_sb[ch][:, ew * W : (ew + 1) * W],
                                start=(ch == 0),
                                stop=(ch == PAIRS - 1),
                            )
                        o_sb = o_pool.tile([P, W], F32, tag="osb")
                        nc.vector.tensor_add(
                            o_sb[:], o_ps[:], bpr_t[:, ew * W : (ew + 1) * W]
                        )
                        nc.sync.dma_start(
                            out[tb * P : (tb + 1) * P, ew * W : (ew + 1) * W],
                            o_sb[:],
                        )
                    return emit
                return [unit(i, ew) for i in range(4) for ew in range(C // W)]

            yt_prev = None
            carry = {}  # units deferred into the NEXT window's plan
            for w in range(NW):
                nkb = 4 * w + 4
                plan = {h: [] for h in range(HL)}
                for h, us in carry.items():
                    plan[h].extend(us)
                carry = {}
                if yt_prev is not None:
                    cps = cproj_units(w - 1, yt_prev)
                    for h, u in zip((0, 1, 2, 3, 4, 5, 6, 7), cps):
                        plan[h].append(u)
                if w + 1 < NW:
                    qkv = qkv_window_units(w + 1)
                    if w + 1 == NW - 1:
                        # Rebalance for the Act-bound last window: its qk
                        # units run just before the pair that needs them,
                        # giving PE filler where Act is the bottleneck. V
                        # units stay in this window (all V blocks are
                        # needed by every last-window head).
                        qf = {j: u for j, u in enumerate(qkv.pop("flat"))}
                        plan[5].extend([qf[8], qf[9]])     # v0, v1
                        plan[6].extend([qf[10], qf[11]])   # v2, v3
                        plan[7].extend([qf[0], qf[4]])     # qk pair 0
                        carry = {
                            1: [qf[1], qf[5]],
                            3: [qf[2], qf[6]],
                            5: [qf[3], qf[7]],
                        }
                    else:
                        for h, us in qkv.items():
                            if h != "flat":
                                plan[h].extend(us)
                yt_w = [
                    yt_pool.tile([P, W], F32R, tag=f"yt{pr}", bufs=2,
                                 name=f"yt{pr}_{w}")
                    for pr in range(PAIRS)
                ]
                for h in range(HL):
                    pr, sub = h // 2, h % 2
                    QT = qt_sb[pr]
                    KT = kt_sb[pr]
                    y_ps = psum.tile([65, W], F32, tag="y", bufs=2,
                                     name=f"y{w}_{h}")
                    pending = []

                    def emit_scores(kb):
                        if kb < 4 * w:
                            cs, mk = 0, None
                        else:
                            i = kb - 4 * w
                            cs = (0, P, 2 * P, 2 * P)[i]
                            mk = i
                        s_ps = psum.tile([P, W], F32, tag="mm",
                                         name=f"s{w}_{h}_{kb}")
                        at = attn_pool.tile([P, W], F32R, tag="attn")
                        nc.tensor.matmul(
                            s_ps[:, cs:W],
                            KT[sub * D : (sub + 1) * D, kb * P : (kb + 1) * P],
                            QT[sub * D : (sub + 1) * D, cs:W],
                            start=True,
                            stop=True,
                        )
                        nc.scalar.activation(
                            at[:, cs:W], s_ps[:, cs:W], EXP,
                            scale=1.0 / np.sqrt(D),
                        )
                        if mk is not None:
                            if mk < 3:
                                nc.vector.tensor_mul(
                                    at[:, mk * P : (mk + 1) * P],
                                    at[:, mk * P : (mk + 1) * P],
                                    tri_t[:],
                                )
                            else:
                                nc.vector.tensor_mul(
                                    at[:, 2 * P : W],
                                    at[:, 2 * P : W],
                                    tri2_t[:],
                                )
                        return (kb, cs, at)

                    def emit_av(kb, cs, at):
                        nc.tensor.matmul(
                            y_ps[:, cs:W],
                            v_sb[:, (h * KB + kb) * 65 : (h * KB + kb + 1) * 65],
                            at[:, cs:W],
                            start=(kb == 0),
                            stop=(kb == nkb - 1),
                        )

                    for kb in range(nkb):
                        pending.append(emit_scores(kb))
                        if len(pending) > LAG:
                            emit_av(*pending.pop(0))
                    for item in pending:
                        emit_av(*item)

                    # softmax normalization: divide y rows by the denominator
                    # accumulated in PSUM row 64.
                    rc = norm_pool.tile([1, W], F32R, tag="recip",
                                        name=f"rc{w}_{h}")
                    nc.vector.reciprocal(rc[:], y_ps[64:65, :])
                    bc_ps = psum.tile([D, W], F32, tag="bc", bufs=1,
                                      name=f"bc{w}_{h}")
                    nc.tensor.matmul(
                        bc_ps[:], ones_row[0:1, 0:D], rc[:],
                        start=True, stop=True,
                    )
                    pbc = norm_pool.tile([D, W], F32, tag="pbc",
                                         name=f"pbc{w}_{h}")
                    nc.vector.tensor_copy(pbc[:], bc_ps[:])
                    nc.vector.tensor_mul(
                        yt_w[pr][sub * D : (sub + 1) * D, :],
                        y_ps[0:D, :],
                        pbc[:],
                    )

                    # interleaved PE filler: previous window's c_proj groups
                    # and next window's QKV units keep PE fed while Act
                    # works through this window's exp backlog.
                    for unit in plan[h]:
                        unit()

                yt_prev = yt_w

            for unit in cproj_units(NW - 1, yt_prev):
                unit()

    nc.compile()
    _CACHE["nc"] = nc
    return nc


def make_in_maps(x, w_attn, b_attn, w_proj, b_proj):
    """Host-side sharding: per-core input dict."""
    x = np.ascontiguousarray(np.asarray(x, dtype=np.float32))
    w_attn = np.asarray(w_attn, dtype=np.float32)
    b_attn = np.asarray(b_attn, dtype=np.float32)
    w_proj = np.asarray(w_proj, dtype=np.float32)
    b_proj = np.asarray(b_proj, dtype=np.float32)

    trimask = np.triu(np.ones((P, P), dtype=np.float32))  # [k, q]: 1 if q >= k
    trimask2 = np.concatenate(
        [np.zeros((P, P), dtype=np.float32), trimask], axis=1
    )
    in_maps = []
    for c in range(N_CORES):
        b = c // 2
        g = c % 2
        h0 = g * HL
        # Q/K columns arranged pair-wise: [q(h0) q(h0+1) | q(h0+2) ... | k(...)]
        qcols = np.arange(h0 * D, (h0 + HL) * D)
        kcols = C + qcols
        wqk = np.concatenate(
            [w_attn[:, qcols], w_attn[:, kcols]], axis=1
        )  # [C, 1024]
        bqk_flat = np.concatenate([b_attn[qcols], b_attn[kcols]])  # [1024]
        bqk = np.ascontiguousarray(bqk_flat.reshape(2 * PAIRS, P).T)  # [128, 8]
        vcols = 2 * C + np.arange(h0 * D, (h0 + HL) * D)
        wv = np.ascontiguousarray(w_attn[:, vcols])  # [C, 512]
        bv = np.broadcast_to(b_attn[vcols], (P, HL * D)).copy()
        wp = np.ascontiguousarray(w_proj[h0 * D : (h0 + HL) * D, :])  # [512, C]
        if g == 0:
            bpr = np.broadcast_to(b_proj, (P, C)).copy()
        else:
            bpr = np.zeros((P, C), dtype=np.float32)
        in_maps.append(
            {
                "xt": np.ascontiguousarray(x[b].T),  # [C, T]
                "wqk": wqk,
                "wv": wv,
                "bqk": bqk,
                "bv": bv,
                "wp": wp,
                "bpr": bpr,
                "trimask": trimask,
                "trimask2": trimask2,
                "onesd": np.ones((P, P), dtype=np.float32),
            }
        )
    return in_maps


def kernel(x, w_attn, b_attn, w_proj, b_proj, _trace=False):
    global LAST_RESULTS
    nc = build_nc()
    in_maps = make_in_maps(x, w_attn, b_attn, w_proj, b_proj)
    res = run_bass_kernel_spmd(
        nc, in_maps, list(range(N_CORES)), trace=_trace
    )
    LAST_RESULTS = res
    outs = [res.results[c]["out"] for c in range(N_CORES)]
    y = np.stack([outs[2 * b] + outs[2 * b + 1] for b in range(B)], axis=0)
    return y.astype(np.float32)



# revision 2
# speedup vs baseline: 1.1417x; 1.1417x over previous
"""Causal self-attention (B=4, T=2048, C=1024, H=16) on 8 TRN2 NeuronCores.

Sharding: hybrid batch x head tensor-parallel. Core c handles batch b = c//2
and heads [8*(c%2) : 8*(c%2)+8]. Each core computes QKV for its 8 heads over
its batch, full causal attention for those heads, and a *partial* c_proj
(contribution of its 8 heads to all 2048 tokens of its batch). The host
unshards by summing the two partial outputs of each batch pair; b_proj is
added on-device by the even core of each pair.

v2: all matmul operands in bf16 (host converts inputs), which (a) halves
input DMA, (b) removes the fp32r moving>=256 constraint. The attn@V matmul
is reoriented to out [q, d+1]: lhsT = exp(scores) [k, q-block], rhs = V
[k, d | ones] so each (q-block, k-block) costs 65 PE rows instead of 512,
and the softmax denominator accumulates in PSUM column 64 (per-partition),
making the normalization a native DVE tensor_scalar. The per-head result
[q, d] is transposed back to [d, q] for c_proj with a PE transpose through
a spare PSUM region. Scores for 2 k-blocks share a [128, 1024] PSUM tile so
one Act exp instruction covers both (fewer Act fixed overheads); diagonal
tri-masks run on GpSimd. The 4 q-block attn@V accumulation groups share one
PSUM bank (start on first write, stop on last).
"""

import numpy as np
import ml_dtypes

import concourse.bass as bass
import concourse.mybir as mybir
import concourse.tile as tile
from concourse import bacc
from concourse.bass_utils import run_bass_kernel_spmd

B, T, C = 4, 2048, 1024
H = 16          # total heads
HL = 8          # heads per core
D = 64          # head dim
P = 128
W = 512         # q-window size
NW = T // W     # 4 q windows
KB = T // P     # 16 k blocks
NCHUNK = C // P  # 8 contraction chunks over C
PAIRS = HL // 2  # 4 head-pairs (2 heads per 128-partition tile)
F32 = mybir.dt.float32
BF16 = mybir.dt.bfloat16
EXP = mybir.ActivationFunctionType.Exp
N_CORES = 8
LAG = 3          # scores-group -> attn@V software pipeline depth per head

_CACHE = {}
LAST_RESULTS = None


def build_nc():
    if "nc" in _CACHE:
        return _CACHE["nc"]
    nc = bacc.Bacc(
        "TRN2", target_bir_lowering=False, debug=False, num_devices=N_CORES
    )

    xt = nc.dram_tensor("xt", [C, T], BF16, kind="ExternalInput")
    wqk = nc.dram_tensor("wqk", [C, C], BF16, kind="ExternalInput")
    wv = nc.dram_tensor("wv", [C, HL * D], BF16, kind="ExternalInput")
    bqk = nc.dram_tensor("bqk", [P, 2 * PAIRS], F32, kind="ExternalInput")
    bv = nc.dram_tensor("bv", [P, HL * D], F32, kind="ExternalInput")
    wp = nc.dram_tensor("wp", [HL * D, C], BF16, kind="ExternalInput")
    bpr = nc.dram_tensor("bpr", [P, C], F32, kind="ExternalInput")
    trimask = nc.dram_tensor("trimask", [P, P], BF16, kind="ExternalInput")
    ident = nc.dram_tensor("ident", [P, P], BF16, kind="ExternalInput")
    out = nc.dram_tensor("out", [T, C], F32, kind="ExternalOutput")

    xt_r = xt[:].rearrange("(a p) t -> p a t", p=P)

    with tile.TileContext(nc) as tc, nc.allow_low_precision(
        reason="bf16 matmul operands, fp32 PSUM accumulation"
    ):
        with (
            tc.tile_pool(name="consts", bufs=1) as consts,
            tc.tile_pool(name="waqk", bufs=NCHUNK) as waqk_pool,
            tc.tile_pool(name="wav", bufs=NCHUNK) as wav_pool,
            tc.tile_pool(name="xtw", bufs=1) as xtw_pool,
            tc.tile_pool(name="kt", bufs=1) as kt_pool,
            tc.tile_pool(name="qt", bufs=PAIRS) as qt_pool,
            tc.tile_pool(name="vsb", bufs=1) as v_pool,
            tc.tile_pool(name="attn", bufs=24) as attn_pool,
            tc.tile_pool(name="ynp", bufs=2) as yn_pool,
            tc.tile_pool(name="yt", bufs=1) as yt_pool,
            tc.tile_pool(name="wp_sb", bufs=1) as wp_pool,
            tc.tile_pool(name="osb", bufs=2) as o_pool,
            tc.tile_pool(name="norm", bufs=1) as norm_pool,
            tc.tile_pool(name="psum", space="PSUM", bufs=3) as psum,
        ):
            # ---- const tiles
            bqk_t = consts.tile([P, 2 * PAIRS], F32)
            bv_t = consts.tile([P, HL * D], F32)
            tri_t = consts.tile([P, P], BF16)
            ident_t = consts.tile([P, P], BF16)
            bpr_t = consts.tile([P, C], F32)

            waqk_all = waqk_pool.tile([P, NCHUNK * C], BF16, bufs=1)
            waqk_sb = [
                waqk_all[:, a * C : (a + 1) * C] for a in range(NCHUNK)
            ]
            wav_all = wav_pool.tile([P, NCHUNK * HL * D], BF16, bufs=1)
            wav_sb = [
                wav_all[:, a * HL * D : (a + 1) * HL * D]
                for a in range(NCHUNK)
            ]
            kt_sb = [
                kt_pool.tile([P, T], BF16, tag=f"kt{pr}", name=f"kt{pr}")
                for pr in range(PAIRS)
            ]
            wp_all = wp_pool.tile([P, PAIRS * C], BF16, bufs=1)
            wp_sb = [
                wp_all[:, ch * C : (ch + 1) * C] for ch in range(PAIRS)
            ]
            # V laid out [tok, d] per (head, kblock) as [P, 65] slices
            # (col 64 stays 1.0 so attn@V accumulates softmax denominators).
            v_sb = v_pool.tile([P, HL * KB * 65], BF16)
            v_view = v_sb[:].rearrange("p (h k c) -> p h k c", h=HL, k=KB)
            bv_view = bv_t[:].rearrange("p (h d) -> p h d", h=HL)

            # ---- DMA kickoff, window-0 critical path first: Q-half weight
            # chunks interleaved with xt window-0 chunks, then K halves,
            # then wv; everything else after.
            xtw_tiles = {}

            def xtw_get(w):
                if w not in xtw_tiles:
                    t = xtw_pool.tile([P, NCHUNK * W], BF16, tag="xtw",
                                      name=f"xtw{w}")
                    tv = t[:].rearrange("p (a t) -> p a t", a=NCHUNK)
                    if w == 0:
                        # graduated granularity: early chunks fine-grained
                        # so window-0 QKV trails the stream, later chunks
                        # coarse to save HWDGE descriptor-generation slots
                        for lo, hi in ((0, 2), (2, 4), (4, 8)):
                            nc.sync.dma_start(
                                tv[:, lo:hi, :],
                                xt_r[:, lo:hi, w * W : (w + 1) * W],
                            )
                    else:
                        nc.sync.dma_start(
                            tv[:, :, :], xt_r[:, :, w * W : (w + 1) * W]
                        )
                    xtw_tiles[w] = t
                return xtw_tiles[w]

            # DMA kickoff: wqk/xt(w0)/wv chunk-wise so window-0 QKV can
            # trail the stream on parallel DMA engines; the rest batched
            # (HWDGE descriptor generation is ~650ns per dma_start).
            wqk_r = wqk[:].rearrange("(a p) c -> p a c", p=P)
            waqk_av = waqk_all[:].rearrange("p (a c) -> p a c", a=NCHUNK)
            wav_av = wav_all[:].rearrange("p (a c) -> p a c", a=NCHUNK)
            wv_r = wv[:].rearrange("(a p) c -> p a c", p=P)

            for lo, hi in ((0, 1), (1, 2), (2, 4), (4, 8)):
                nc.sync.dma_start(waqk_av[:, lo:hi, :], wqk_r[:, lo:hi, :])
            nc.sync.dma_start(bqk_t[:], bqk[:])
            xtw_get(0)
            for lo, hi in ((0, 4), (4, 8)):
                nc.sync.dma_start(wav_av[:, lo:hi, :], wv_r[:, lo:hi, :])
            nc.sync.dma_start(bv_t[:], bv[:])
            # ones column (col 64 of every [P, 65] V slice)
            nc.gpsimd.memset(v_view[:, :, :, 64:65], 1.0)
            nc.sync.dma_start(tri_t[:], trimask[:])
            nc.sync.dma_start(ident_t[:], ident[:])
            nc.sync.dma_start(
                wp_all[:].rearrange("p (a c) -> p a c", a=PAIRS),
                wp[:].rearrange("(a p) c -> p a c", p=P),
            )
            nc.sync.dma_start(bpr_t[:], bpr[:])

            qt_by_w = {}

            def emit_qk_copy(j, qk_ps, w):
                # move PSUM -> SBUF (bf16) with the per-qk-column bias added
                if j < PAIRS:
                    qts = qt_by_w.setdefault(w, [None] * PAIRS)
                    qts[j] = qt_pool.tile(
                        [P, W], BF16, tag=f"qt{j}", bufs=2, name=f"qt{j}_{w}"
                    )
                    dest = qts[j][:]
                else:
                    dest = kt_sb[j - PAIRS][:, w * W : (w + 1) * W]
                nc.vector.tensor_scalar(
                    out=dest,
                    in0=qk_ps[:],
                    scalar1=bqk_t[:, j : j + 1],
                    scalar2=None,
                    op0=mybir.AluOpType.add,
                )

            def emit_v_add(i, v_ps, w):
                tb = 4 * w + i
                nc.vector.tensor_add(
                    v_view[:, :, tb, 0:D],
                    v_ps[:].rearrange("p (h d) -> p h d", h=HL),
                    bv_view[:, :, :],
                )

            def qkv_window0_pair(pr):
                # window-0 Q+K projection for pair pr, chunk-major across
                # the two fl banks so PE trails the per-chunk DMA stream.
                def emit():
                    xtw = xtw_get(0)
                    js = (pr, PAIRS + pr)
                    qk_ps = [
                        psum.tile([P, W], F32, tag="fl", bufs=2,
                                  name=f"qk0_{j}")
                        for j in js
                    ]
                    for a in range(NCHUNK):
                        for t, j in enumerate(js):
                            nc.tensor.matmul(
                                qk_ps[t][:],
                                waqk_sb[a][:, j * P : (j + 1) * P],
                                xtw[:, a * W : (a + 1) * W],
                                start=(a == 0),
                                stop=(a == NCHUNK - 1),
                            )
                    for t, j in enumerate(js):
                        emit_qk_copy(j, qk_ps[t], 0)
                return emit

            def qkv_window_units(w):
                # windows >= 1: inputs already resident, one closure per
                # projection unit so the caller can interleave them between
                # attention heads as PE filler.
                xtw = xtw_get(w)

                def qk_unit(j):
                    def emit():
                        qk_ps = psum.tile([P, W], F32, tag="fl", bufs=2,
                                          name=f"qk{w}_{j}")
                        for a in range(NCHUNK):
                            nc.tensor.matmul(
                                qk_ps[:],
                                waqk_sb[a][:, j * P : (j + 1) * P],
                                xtw[:, a * W : (a + 1) * W],
                                start=(a == 0),
                                stop=(a == NCHUNK - 1),
                            )
                        emit_qk_copy(j, qk_ps, w)
                    return emit

                def v_unit(i):
                    def emit():
                        v_ps = psum.tile([P, W], F32, tag="fl", bufs=2,
                                         name=f"v{w}_{i}")
                        for a in range(NCHUNK):
                            nc.tensor.matmul(
                                v_ps[:],
                                xtw[:, a * W + i * P : a * W + (i + 1) * P],
                                wav_sb[a][:],
                                start=(a == 0),
                                stop=(a == NCHUNK - 1),
                            )
                        emit_v_add(i, v_ps, w)
                    return emit

                if w == 0:
                    # window 0: Q and K of a pair are fused chunk-major so
                    # PE trails the weight/x DMA stream; ensure_qk marks
                    # both j and PAIRS+j done via the shared closure.
                    pairs = [qkv_window0_pair(pr) for pr in range(PAIRS)]
                    return {
                        "qk": pairs + pairs,
                        "v": [v_unit(i) for i in range(4)],
                    }
                return {
                    "qk": [qk_unit(j) for j in range(2 * PAIRS)],
                    "v": [v_unit(i) for i in range(4)],
                }

            def cproj_units(w, yt_tiles):
                # partial c_proj of a finished window's 4 token blocks, one
                # closure per (tb, ew) group so they can interleave as PE
                # filler inside the next window's attention. The two ew
                # halves of a token block share one [P, C] staging tile and
                # one output DMA (fired by whichever half finishes second).
                osb_state = {}

                def unit(i, ew):
                    def emit():
                        tb = 4 * w + i
                        o_ps = psum.tile([P, W], F32, tag="fl", bufs=2,
                                         name=f"o{tb}_{ew}")
                        for ch in range(PAIRS):
                            nc.tensor.matmul(
                                o_ps[:],
                                yt_tiles[ch][:, i * P : (i + 1) * P],
                                wp_sb[ch][:, ew * W : (ew + 1) * W],
                                start=(ch == 0),
                                stop=(ch == PAIRS - 1),
                            )
                        if tb not in osb_state:
                            osb_state[tb] = o_pool.tile(
                                [P, C], F32, tag="osb", name=f"osb{tb}"
                            )
                        o_sb = osb_state[tb]
                        nc.vector.tensor_add(
                            o_sb[:, ew * W : (ew + 1) * W],
                            o_ps[:],
                            bpr_t[:, ew * W : (ew + 1) * W],
                        )
                        if w == NW - 1:
                            # tail: per-half DMAs transfer in parallel
                            nc.sync.dma_start(
                                out[tb * P : (tb + 1) * P,
                                    ew * W : (ew + 1) * W],
                                o_sb[:, ew * W : (ew + 1) * W],
                            )
                            return
                        done = osb_state.setdefault((tb, "n"), [])
                        done.append(ew)
                        if len(done) == 2:
                            nc.sync.dma_start(
                                out[tb * P : (tb + 1) * P, :], o_sb[:]
                            )
                    return emit
                return [unit(i, ew) for i in range(4) for ew in range(C // W)]

            def emit_scores_group(w, h, g):
                # scores for kbs 2g, 2g+1 in one [128, 1024] PSUM tile; one
                # exp covers both (garbage cols in the diagonal groups are
                # exp'd but never read).
                pr, sub = h // 2, h % 2
                QT = qt_by_w[w][pr]
                KT = kt_sb[pr]
                s_ps = psum.tile([P, 2 * W], F32, tag="s", bufs=2,
                                 name=f"s{w}_{h}_{g}")
                at = attn_pool.tile([P, 2 * W], BF16, tag="attn",
                                    name=f"at{w}_{h}_{g}")
                css = []
                for i in range(2):
                    kb = 2 * g + i
                    cs = max(0, kb - 4 * w) * P
                    css.append(cs)
                    nc.tensor.matmul(
                        s_ps[:, i * W + cs : (i + 1) * W],
                        KT[sub * D : (sub + 1) * D, kb * P : (kb + 1) * P],
                        QT[sub * D : (sub + 1) * D, cs:W],
                        start=True,
                        stop=True,
                    )
                lo = css[0]  # first valid col of the 2-slot span
                nc.scalar.activation(
                    at[:, lo:], s_ps[:, lo:], EXP, scale=1.0 / np.sqrt(D)
                )
                # tri-mask the diagonal blocks on GpSimd
                for i in range(2):
                    kb = 2 * g + i
                    qb = kb - 4 * w
                    if 0 <= qb < 4:
                        nc.gpsimd.tensor_mul(
                            at[:, i * W + qb * P : i * W + (qb + 1) * P],
                            at[:, i * W + qb * P : i * W + (qb + 1) * P],
                            tri_t[:],
                        )
                return at

            # ---------------- global software pipeline ----------------
            # One stream of scores+exp "production" runs ahead of the
            # attn@V "consumption" stream, buffered by the at-tile store,
            # so the Act engine's idle early capacity absorbs exp work
            # that would otherwise gate the causally-heavy late windows.
            # A coarse compile-time time model decides, at each emission
            # step, between producing, consuming, and PE filler
            # (QKV/c_proj units) so the in-order PE stream rarely waits.
            MM = 1.0 / 2.4  # ns per PE cycle
            STORE_CAP = 16

            def ngrp(w):
                return 2 * w + 2

            qkv_state = {}

            def get_qkv(w):
                if w not in qkv_state:
                    u = qkv_window_units(w)
                    qkv_state[w] = {
                        "qk": u["qk"], "v": u["v"],
                        "qk_done": [False] * (2 * PAIRS),
                        "v_done": [False] * 4,
                    }
                return qkv_state[w]

            # time model state (ns)
            tm = {"pe": 0.0, "act": 0.0}
            sband = [0.0, 0.0]
            n_prod = 0

            def ensure_qk(w, j):
                st = get_qkv(w)
                if not st["qk_done"][j]:
                    st["qk"][j]()
                    st["qk_done"][j] = True
                    tm["pe"] += 1707
                    if w == 0:
                        # shared Q+K pair closure covers both slots
                        st["qk_done"][(j + PAIRS) % (2 * PAIRS)] = True
                        tm["pe"] += 1707

            def ensure_v(w, i):
                st = get_qkv(w)
                if not st["v_done"][i]:
                    st["v"][i]()
                    st["v_done"][i] = True
                    tm["pe"] += 1707

            def drain_qkv_one(wmax):
                for w in range(0, wmax + 1):
                    st = get_qkv(w)
                    for j in range(2 * PAIRS):
                        if not st["qk_done"][j]:
                            ensure_qk(w, j)
                            return True
                    for i in range(4):
                        if not st["v_done"][i]:
                            ensure_v(w, i)
                            return True
                return False

            store = {}
            group_ready = {}
            cproj_pending = []  # (w, [unit closures])

            def run_cproj_unit():
                if not cproj_pending:
                    return False
                w, units = cproj_pending[0]
                units.pop(0)()
                tm["pe"] += 853
                if not units:
                    cproj_pending.pop(0)
                return True

            def force_cproj(w):
                while cproj_pending and cproj_pending[0][0] <= w:
                    run_cproj_unit()

            all_groups = [
                (w, h, g)
                for w in range(NW)
                for h in range(HL)
                for g in range(ngrp(w))
            ]
            prod_pos = 0

            def can_produce():
                if prod_pos >= len(all_groups) or len(store) >= STORE_CAP:
                    return False
                return True

            def produce():
                nonlocal prod_pos, n_prod
                w, h, g = all_groups[prod_pos]
                prod_pos += 1
                pr = h // 2
                diag = 2 * g + 1 >= 4 * w
                ensure_qk(w, pr)
                if diag:
                    ensure_qk(w, PAIRS + pr)
                # time model: scores wait on the s-bank freed by the exp
                # two productions back, then Act runs the exp FIFO.
                scores_dur = sum(
                    (W - max(0, 2 * g + i - 4 * w) * P) * MM
                    for i in range(2)
                )
                lo = max(0, 2 * g - 4 * w) * P
                exp_dur = (2 * W - lo) * 0.833 + 190
                tm["pe"] = max(tm["pe"], sband[n_prod % 2]) + scores_dur
                act_done = max(tm["act"], tm["pe"] + 300) + exp_dur
                tm["act"] = act_done
                sband[n_prod % 2] = act_done
                n_prod += 1
                at = emit_scores_group(w, h, g)
                store[(w, h, g)] = at
                group_ready[(w, h, g)] = act_done + (650 if diag else 250)

            yt_by_w = {}
            for w in range(NW):
                nkb = 4 * w + 4
                yt_w = [
                    yt_pool.tile([P, W], BF16, tag=f"yt{pr}", bufs=2,
                                 name=f"yt{pr}_{w}")
                    for pr in range(PAIRS)
                ]
                yt_by_w[w] = yt_w
                for h in range(HL):
                    pr, sub = h // 2, h % 2
                    if w >= 2 and h == 0:
                        # yt tiles are double-buffered: window w's writes
                        # reuse window w-2's buffers, so c_proj(w-2) must
                        # be fully emitted first.
                        force_cproj(w - 2)
                    # one PSUM bank for the 4 q-block attn@V groups (cols
                    # qb*65..qb*65+64) + spare region for the transposes.
                    y_ps = psum.tile([P, W], F32, tag="y", bufs=2,
                                     name=f"y{w}_{h}")
                    n_av = 0
                    n_av_total = sum(
                        4 - max(0, kb - 4 * w) for kb in range(nkb)
                    )

                    def emit_av_group(g, at):
                        nonlocal n_av
                        for i in range(2):
                            kb = 2 * g + i
                            if kb >= 4 * w:
                                ensure_v(w, kb - 4 * w)
                            for qb in range(max(0, kb - 4 * w), 4):
                                nc.tensor.matmul(
                                    y_ps[:, qb * 65 : qb * 65 + 65],
                                    at[:, i * W + qb * P : i * W + (qb + 1) * P],
                                    v_sb[:, (h * KB + kb) * 65
                                         : (h * KB + kb + 1) * 65],
                                    start=(n_av == 0),
                                    stop=(n_av == n_av_total - 1),
                                    skip_group_check=True,
                                )
                                n_av += 1

                    for g in range(ngrp(w)):
                        key = (w, h, g)
                        while True:
                            # produce-first: keep ~2 exps in flight on Act
                            # (the s-bank double-buffer ceiling) so the Act
                            # engine never starves and the store absorbs
                            # its early idle capacity.
                            if can_produce() and tm["act"] - tm["pe"] < 1800:
                                produce()
                                continue
                            if key in store and group_ready[key] <= tm["pe"]:
                                break
                            if run_cproj_unit():
                                continue
                            if drain_qkv_one(min(w + 1, NW - 1)):
                                continue
                            if can_produce():
                                produce()
                                continue
                            if key in store:
                                tm["pe"] = group_ready[key]
                                break
                            # store full of future groups but key missing
                            # cannot happen (FIFO production); stall-produce
                            produce()
                        at = store.pop(key)
                        av_dur = sum(
                            (4 - max(0, 2 * g + i - 4 * w)) * 65 * MM
                            for i in range(2)
                        )
                        tm["pe"] += av_dur
                        emit_av_group(g, at)

                    # normalize (per-partition scalar) + PE transpose back
                    # to [d, q] through spare regions of the y bank.
                    yn = yn_pool.tile([P, 4 * D], BF16, tag="yn",
                                      name=f"yn{w}_{h}")
                    rc = norm_pool.tile([P, 4], F32, tag="rc", bufs=2,
                                        name=f"rc{w}_{h}")
                    y_qv = y_ps[:, 0 : 4 * 65].rearrange(
                        "p (q c) -> p q c", c=65
                    )
                    nc.vector.reciprocal(rc[:], y_qv[:, :, 64])
                    # the very tail runs on Act (idle once exps are done)
                    # to shorten the last heads' serial normalize chain.
                    on_act = w == NW - 1 and h >= HL - 2
                    for qb in range(4):
                        if on_act:
                            nc.scalar.activation(
                                yn[:, qb * D : (qb + 1) * D],
                                y_ps[:, qb * 65 : qb * 65 + 64],
                                mybir.ActivationFunctionType.Copy,
                                scale=rc[:, qb : qb + 1],
                            )
                        else:
                            nc.vector.tensor_scalar(
                                out=yn[:, qb * D : (qb + 1) * D],
                                in0=y_ps[:, qb * 65 : qb * 65 + 64],
                                scalar1=rc[:, qb : qb + 1],
                                scalar2=None,
                                op0=mybir.AluOpType.mult,
                            )
                    t_ps = [
                        y_ps[0:D, 384:448].bitcast(BF16),
                        y_ps[0:D, 448:512].bitcast(BF16),
                    ]
                    for qb in range(4):
                        tp = t_ps[qb % 2]
                        nc.tensor.transpose(
                            tp, yn[:, qb * D : (qb + 1) * D], ident_t[:]
                        )
                        dst = yt_w[pr][sub * D : (sub + 1) * D,
                                       qb * P : (qb + 1) * P]
                        if on_act:
                            nc.scalar.copy(dst, tp)
                        else:
                            nc.vector.tensor_copy(dst, tp)
                    tm["pe"] += 400

                cproj_pending.append((w, cproj_units(w, yt_w)))

            # drain whatever filler remains
            while run_cproj_unit():
                pass

    nc.compile()
    _CACHE["nc"] = nc
    return nc


def make_in_maps(x, w_attn, b_attn, w_proj, b_proj):
    """Host-side sharding: per-core input dict (matmul operands in bf16)."""
    x = np.ascontiguousarray(np.asarray(x, dtype=np.float32))
    w_attn = np.asarray(w_attn, dtype=np.float32)
    b_attn = np.asarray(b_attn, dtype=np.float32)
    w_proj = np.asarray(w_proj, dtype=np.float32)
    b_proj = np.asarray(b_proj, dtype=np.float32)

    def bf(a):
        return np.ascontiguousarray(a).astype(ml_dtypes.bfloat16)

    trimask = bf(np.triu(np.ones((P, P), np.float32)))  # [k, q]: 1 if q >= k
    ident = bf(np.eye(P, dtype=np.float32))
    in_maps = []
    for c in range(N_CORES):
        b = c // 2
        g = c % 2
        h0 = g * HL
        # Q/K columns arranged pair-wise: [q(h0) q(h0+1) | q(h0+2) ... | k(...)]
        qcols = np.arange(h0 * D, (h0 + HL) * D)
        kcols = C + qcols
        wqk = np.concatenate(
            [w_attn[:, qcols], w_attn[:, kcols]], axis=1
        )  # [C, 1024]
        bqk_flat = np.concatenate([b_attn[qcols], b_attn[kcols]])  # [1024]
        bqk = np.ascontiguousarray(bqk_flat.reshape(2 * PAIRS, P).T)  # [128, 8]
        vcols = 2 * C + np.arange(h0 * D, (h0 + HL) * D)
        wv = np.ascontiguousarray(w_attn[:, vcols])  # [C, 512]
        bv = np.broadcast_to(b_attn[vcols], (P, HL * D)).copy()
        wp = np.ascontiguousarray(w_proj[h0 * D : (h0 + HL) * D, :])  # [512, C]
        if g == 0:
            bpr = np.broadcast_to(b_proj, (P, C)).copy()
        else:
            bpr = np.zeros((P, C), dtype=np.float32)
        in_maps.append(
            {
                "xt": bf(x[b].T),  # [C, T]
                "wqk": bf(wqk),
                "wv": bf(wv),
                "bqk": bqk,
                "bv": bv,
                "wp": bf(wp),
                "bpr": bpr,
                "trimask": trimask,
                "ident": ident,
            }
        )
    return in_maps


def kernel(x, w_attn, b_attn, w_proj, b_proj, _trace=False):
    global LAST_RESULTS
    nc = build_nc()
    in_maps = make_in_maps(x, w_attn, b_attn, w_proj, b_proj)
    res = run_bass_kernel_spmd(
        nc, in_maps, list(range(N_CORES)), trace=_trace
    )
    LAST_RESULTS = res
    outs = [res.results[c]["out"] for c in range(N_CORES)]
    y = np.stack([outs[2 * b] + outs[2 * b + 1] for b in range(B)], axis=0)
    return y.astype(np.float32)


# revision 5
# speedup vs baseline: 1.1535x; 1.0104x over previous
"""Causal self-attention (B=4, T=2048, C=1024, H=16) on 8 TRN2 NeuronCores.

Sharding: hybrid batch x head tensor-parallel. Core c handles batch b = c//2
and heads [8*(c%2) : 8*(c%2)+8]. Each core computes QKV for its 8 heads over
its batch, full causal attention for those heads, and a *partial* c_proj
(contribution of its 8 heads to all 2048 tokens of its batch). The host
unshards by summing the two partial outputs of each batch pair; b_proj is
added on-device by the even core of each pair.

v2: all matmul operands in bf16 (host converts inputs), which (a) halves
input DMA, (b) removes the fp32r moving>=256 constraint. The attn@V matmul
is reoriented to out [q, d+1]: lhsT = exp(scores) [k, q-block], rhs = V
[k, d | ones] so each (q-block, k-block) costs 65 PE rows instead of 512,
and the softmax denominator accumulates in PSUM column 64 (per-partition),
making the normalization a native DVE tensor_scalar. The per-head result
[q, d] is transposed back to [d, q] for c_proj with a PE transpose through
a spare PSUM region. Scores for 2 k-blocks share a [128, 1024] PSUM tile so
one Act exp instruction covers both (fewer Act fixed overheads); diagonal
tri-masks run on GpSimd. The 4 q-block attn@V accumulation groups share one
PSUM bank (start on first write, stop on last).
"""

import numpy as np
import ml_dtypes

import concourse.bass as bass
import concourse.mybir as mybir
import concourse.tile as tile
from concourse import bacc
from concourse.bass_utils import run_bass_kernel_spmd

B, T, C = 4, 2048, 1024
H = 16          # total heads
HL = 8          # heads per core
D = 64          # head dim
P = 128
W = 512         # q-window size
NW = T // W     # 4 q windows
KB = T // P     # 16 k blocks
NCHUNK = C // P  # 8 contraction chunks over C
PAIRS = HL // 2  # 4 head-pairs (2 heads per 128-partition tile)
F32 = mybir.dt.float32
BF16 = mybir.dt.bfloat16
EXP = mybir.ActivationFunctionType.Exp
N_CORES = 8
LAG = 3          # scores-group -> attn@V software pipeline depth per head

_CACHE = {}
LAST_RESULTS = None


def build_nc():
    if "nc" in _CACHE:
        return _CACHE["nc"]
    nc = bacc.Bacc(
        "TRN2", target_bir_lowering=False, debug=False, num_devices=N_CORES
    )

    xt = nc.dram_tensor("xt", [C, T], BF16, kind="ExternalInput")
    wqk = nc.dram_tensor("wqk", [C, C], BF16, kind="ExternalInput")
    wv = nc.dram_tensor("wv", [C, HL * D], BF16, kind="ExternalInput")
    bqk = nc.dram_tensor("bqk", [P, 2 * PAIRS], F32, kind="ExternalInput")
    bv = nc.dram_tensor("bv", [P, HL * D], F32, kind="ExternalInput")
    wp = nc.dram_tensor("wp", [HL * D, C], BF16, kind="ExternalInput")
    bpr = nc.dram_tensor("bpr", [P, C], F32, kind="ExternalInput")
    trimask = nc.dram_tensor("trimask", [P, P], BF16, kind="ExternalInput")
    ident = nc.dram_tensor("ident", [P, P], BF16, kind="ExternalInput")
    out = nc.dram_tensor("out", [T, C], F32, kind="ExternalOutput")

    xt_r = xt[:].rearrange("(a p) t -> p a t", p=P)

    with tile.TileContext(nc) as tc, nc.allow_low_precision(
        reason="bf16 matmul operands, fp32 PSUM accumulation"
    ):
        with (
            tc.tile_pool(name="consts", bufs=1) as consts,
            tc.tile_pool(name="waqk", bufs=NCHUNK) as waqk_pool,
            tc.tile_pool(name="wav", bufs=NCHUNK) as wav_pool,
            tc.tile_pool(name="xtw", bufs=1) as xtw_pool,
            tc.tile_pool(name="kt", bufs=1) as kt_pool,
            tc.tile_pool(name="qt", bufs=PAIRS) as qt_pool,
            tc.tile_pool(name="vsb", bufs=1) as v_pool,
            tc.tile_pool(name="attn", bufs=24) as attn_pool,
            tc.tile_pool(name="ynp", bufs=2) as yn_pool,
            tc.tile_pool(name="yt", bufs=1) as yt_pool,
            tc.tile_pool(name="wp_sb", bufs=1) as wp_pool,
            tc.tile_pool(name="osb", bufs=2) as o_pool,
            tc.tile_pool(name="norm", bufs=1) as norm_pool,
            tc.tile_pool(name="psum", space="PSUM", bufs=3) as psum,
        ):
            # ---- const tiles
            bqk_t = consts.tile([P, 2 * PAIRS], F32)
            bv_t = consts.tile([P, HL * D], F32)
            tri_t = consts.tile([P, P], BF16)
            ident_t = consts.tile([P, P], BF16)
            bpr_t = consts.tile([P, C], F32)

            waqk_all = waqk_pool.tile([P, NCHUNK * C], BF16, bufs=1)
            waqk_sb = [
                waqk_all[:, a * C : (a + 1) * C] for a in range(NCHUNK)
            ]
            wav_all = wav_pool.tile([P, NCHUNK * HL * D], BF16, bufs=1)
            wav_sb = [
                wav_all[:, a * HL * D : (a + 1) * HL * D]
                for a in range(NCHUNK)
            ]
            kt_sb = [
                kt_pool.tile([P, T], BF16, tag=f"kt{pr}", name=f"kt{pr}")
                for pr in range(PAIRS)
            ]
            wp_all = wp_pool.tile([P, PAIRS * C], BF16, bufs=1)
            wp_sb = [
                wp_all[:, ch * C : (ch + 1) * C] for ch in range(PAIRS)
            ]
            # V laid out [tok, d] per (head, kblock) as [P, 65] slices
            # (col 64 stays 1.0 so attn@V accumulates softmax denominators).
            v_sb = v_pool.tile([P, HL * KB * 65], BF16)
            v_view = v_sb[:].rearrange("p (h k c) -> p h k c", h=HL, k=KB)
            bv_view = bv_t[:].rearrange("p (h d) -> p h d", h=HL)

            # ---- DMA kickoff, window-0 critical path first: Q-half weight
            # chunks interleaved with xt window-0 chunks, then K halves,
            # then wv; everything else after.
            xtw_tiles = {}

            def xtw_get(w):
                if w not in xtw_tiles:
                    t = xtw_pool.tile([P, NCHUNK * W], BF16, tag="xtw",
                                      name=f"xtw{w}")
                    tv = t[:].rearrange("p (a t) -> p a t", a=NCHUNK)
                    if w == 0:
                        # graduated granularity: early chunks fine-grained
                        # so window-0 QKV trails the stream, later chunks
                        # coarse to save HWDGE descriptor-generation slots
                        for lo, hi in ((0, 2), (2, 4), (4, 8)):
                            nc.sync.dma_start(
                                tv[:, lo:hi, :],
                                xt_r[:, lo:hi, w * W : (w + 1) * W],
                            )
                    else:
                        nc.sync.dma_start(
                            tv[:, :, :], xt_r[:, :, w * W : (w + 1) * W]
                        )
                    xtw_tiles[w] = t
                return xtw_tiles[w]

            # DMA kickoff: wqk/xt(w0)/wv chunk-wise so window-0 QKV can
            # trail the stream on parallel DMA engines; the rest batched
            # (HWDGE descriptor generation is ~650ns per dma_start).
            wqk_r = wqk[:].rearrange("(a p) c -> p a c", p=P)
            waqk_av = waqk_all[:].rearrange("p (a c) -> p a c", a=NCHUNK)
            wav_av = wav_all[:].rearrange("p (a c) -> p a c", a=NCHUNK)
            wv_r = wv[:].rearrange("(a p) c -> p a c", p=P)

            for lo, hi in ((0, 1), (1, 2), (2, 4), (4, 8)):
                nc.sync.dma_start(waqk_av[:, lo:hi, :], wqk_r[:, lo:hi, :])
            nc.sync.dma_start(bqk_t[:], bqk[:])
            xtw_get(0)
            for lo, hi in ((0, 4), (4, 8)):
                nc.sync.dma_start(wav_av[:, lo:hi, :], wv_r[:, lo:hi, :])
            nc.sync.dma_start(bv_t[:], bv[:])
            # ones column (col 64 of every [P, 65] V slice)
            nc.gpsimd.memset(v_view[:, :, :, 64:65], 1.0)
            nc.sync.dma_start(tri_t[:], trimask[:])
            nc.sync.dma_start(ident_t[:], ident[:])
            nc.sync.dma_start(
                wp_all[:].rearrange("p (a c) -> p a c", a=PAIRS),
                wp[:].rearrange("(a p) c -> p a c", p=P),
            )
            nc.sync.dma_start(bpr_t[:], bpr[:])

            qt_by_w = {}

            def emit_qk_copy(j, qk_ps, w):
                # move PSUM -> SBUF (bf16) with the per-qk-column bias added
                if j < PAIRS:
                    qts = qt_by_w.setdefault(w, [None] * PAIRS)
                    qts[j] = qt_pool.tile(
                        [P, W], BF16, tag=f"qt{j}", bufs=2, name=f"qt{j}_{w}"
                    )
                    dest = qts[j][:]
                else:
                    dest = kt_sb[j - PAIRS][:, w * W : (w + 1) * W]
                nc.vector.tensor_scalar(
                    out=dest,
                    in0=qk_ps[:],
                    scalar1=bqk_t[:, j : j + 1],
                    scalar2=None,
                    op0=mybir.AluOpType.add,
                )

            def emit_v_add(i, v_ps, w):
                tb = 4 * w + i
                nc.vector.tensor_add(
                    v_view[:, :, tb, 0:D],
                    v_ps[:].rearrange("p (h d) -> p h d", h=HL),
                    bv_view[:, :, :],
                )

            def qkv_window0_pair(pr):
                # window-0 Q+K projection for pair pr, chunk-major across
                # the two fl banks so PE trails the per-chunk DMA stream.
                def emit():
                    xtw = xtw_get(0)
                    js = (pr, PAIRS + pr)
                    qk_ps = [
                        psum.tile([P, W], F32, tag="fl", bufs=2,
                                  name=f"qk0_{j}")
                        for j in js
                    ]
                    for a in range(NCHUNK):
                        for t, j in enumerate(js):
                            nc.tensor.matmul(
                                qk_ps[t][:],
                                waqk_sb[a][:, j * P : (j + 1) * P],
                                xtw[:, a * W : (a + 1) * W],
                                start=(a == 0),
                                stop=(a == NCHUNK - 1),
                            )
                    for t, j in enumerate(js):
                        emit_qk_copy(j, qk_ps[t], 0)
                return emit

            def qkv_window_units(w):
                # windows >= 1: inputs already resident, one closure per
                # projection unit so the caller can interleave them between
                # attention heads as PE filler.
                xtw = xtw_get(w)

                def qk_unit(j):
                    def emit():
                        qk_ps = psum.tile([P, W], F32, tag="fl", bufs=2,
                                          name=f"qk{w}_{j}")
                        for a in range(NCHUNK):
                            nc.tensor.matmul(
                                qk_ps[:],
                                waqk_sb[a][:, j * P : (j + 1) * P],
                                xtw[:, a * W : (a + 1) * W],
                                start=(a == 0),
                                stop=(a == NCHUNK - 1),
                            )
                        emit_qk_copy(j, qk_ps, w)
                    return emit

                def v_unit(i):
                    def emit():
                        v_ps = psum.tile([P, W], F32, tag="fl", bufs=2,
                                         name=f"v{w}_{i}")
                        for a in range(NCHUNK):
                            nc.tensor.matmul(
                                v_ps[:],
                                xtw[:, a * W + i * P : a * W + (i + 1) * P],
                                wav_sb[a][:],
                                start=(a == 0),
                                stop=(a == NCHUNK - 1),
                            )
                        emit_v_add(i, v_ps, w)
                    return emit

                if w == 0:
                    # window 0: Q and K of a pair are fused chunk-major so
                    # PE trails the weight/x DMA stream; ensure_qk marks
                    # both j and PAIRS+j done via the shared closure.
                    pairs = [qkv_window0_pair(pr) for pr in range(PAIRS)]
                    return {
                        "qk": pairs + pairs,
                        "v": [v_unit(i) for i in range(4)],
                    }
                return {
                    "qk": [qk_unit(j) for j in range(2 * PAIRS)],
                    "v": [v_unit(i) for i in range(4)],
                }

            def cproj_units(w, yt_tiles):
                # partial c_proj of a finished window's 4 token blocks, one
                # closure per (tb, ew) group so they can interleave as PE
                # filler inside the next window's attention. The two ew
                # halves of a token block share one [P, C] staging tile and
                # one output DMA (fired by whichever half finishes second).
                osb_state = {}

                def unit(i, ew):
                    def emit():
                        tb = 4 * w + i
                        o_ps = psum.tile([P, W], F32, tag="fl", bufs=2,
                                         name=f"o{tb}_{ew}")
                        for ch in range(PAIRS):
                            nc.tensor.matmul(
                                o_ps[:],
                                yt_tiles[ch][:, i * P : (i + 1) * P],
                                wp_sb[ch][:, ew * W : (ew + 1) * W],
                                start=(ch == 0),
                                stop=(ch == PAIRS - 1),
                            )
                        if tb not in osb_state:
                            osb_state[tb] = o_pool.tile(
                                [P, C], F32, tag="osb", name=f"osb{tb}"
                            )
                        o_sb = osb_state[tb]
                        nc.vector.tensor_add(
                            o_sb[:, ew * W : (ew + 1) * W],
                            o_ps[:],
                            bpr_t[:, ew * W : (ew + 1) * W],
                        )
                        if w == NW - 1:
                            # tail: per-half DMAs transfer in parallel
                            nc.sync.dma_start(
                                out[tb * P : (tb + 1) * P,
                                    ew * W : (ew + 1) * W],
                                o_sb[:, ew * W : (ew + 1) * W],
                            )
                            return
                        done = osb_state.setdefault((tb, "n"), [])
                        done.append(ew)
                        if len(done) == 2:
                            nc.sync.dma_start(
                                out[tb * P : (tb + 1) * P, :], o_sb[:]
                            )
                    return emit
                return [unit(i, ew) for i in range(4) for ew in range(C // W)]

            def emit_scores_group(w, h, g):
                # scores for kbs 2g, 2g+1 in one [128, 1024] PSUM tile; one
                # exp covers both (garbage cols in the diagonal groups are
                # exp'd but never read).
                pr, sub = h // 2, h % 2
                QT = qt_by_w[w][pr]
                KT = kt_sb[pr]
                s_ps = psum.tile([P, 2 * W], F32, tag="s", bufs=2,
                                 name=f"s{w}_{h}_{g}")
                at = attn_pool.tile([P, 2 * W], BF16, tag="attn",
                                    name=f"at{w}_{h}_{g}")
                css = []
                for i in range(2):
                    kb = 2 * g + i
                    cs = max(0, kb - 4 * w) * P
                    css.append(cs)
                    nc.tensor.matmul(
                        s_ps[:, i * W + cs : (i + 1) * W],
                        KT[sub * D : (sub + 1) * D, kb * P : (kb + 1) * P],
                        QT[sub * D : (sub + 1) * D, cs:W],
                        start=True,
                        stop=True,
                    )
                lo = css[0]  # first valid col of the 2-slot span
                nc.scalar.activation(
                    at[:, lo:], s_ps[:, lo:], EXP, scale=1.0 / np.sqrt(D)
                )
                # tri-mask the diagonal blocks on GpSimd
                for i in range(2):
                    kb = 2 * g + i
                    qb = kb - 4 * w
                    if 0 <= qb < 4:
                        nc.gpsimd.tensor_mul(
                            at[:, i * W + qb * P : i * W + (qb + 1) * P],
                            at[:, i * W + qb * P : i * W + (qb + 1) * P],
                            tri_t[:],
                        )
                return at

            # ---------------- global software pipeline ----------------
            # One stream of scores+exp "production" runs ahead of the
            # attn@V "consumption" stream, buffered by the at-tile store,
            # so the Act engine's idle early capacity absorbs exp work
            # that would otherwise gate the causally-heavy late windows.
            # A coarse compile-time time model decides, at each emission
            # step, between producing, consuming, and PE filler
            # (QKV/c_proj units) so the in-order PE stream rarely waits.
            MM = 1.0 / 2.4  # ns per PE cycle
            STORE_CAP = 22

            def ngrp(w):
                return 2 * w + 2

            qkv_state = {}

            def get_qkv(w):
                if w not in qkv_state:
                    u = qkv_window_units(w)
                    qkv_state[w] = {
                        "qk": u["qk"], "v": u["v"],
                        "qk_done": [False] * (2 * PAIRS),
                        "v_done": [False] * 4,
                    }
                return qkv_state[w]

            # time model state (ns)
            tm = {"pe": 0.0, "act": 0.0}
            sband = [0.0, 0.0]
            n_prod = 0

            def ensure_qk(w, j):
                st = get_qkv(w)
                if not st["qk_done"][j]:
                    st["qk"][j]()
                    st["qk_done"][j] = True
                    tm["pe"] += 1707
                    if w == 0:
                        # shared Q+K pair closure covers both slots
                        st["qk_done"][(j + PAIRS) % (2 * PAIRS)] = True
                        tm["pe"] += 1707

            def ensure_v(w, i):
                st = get_qkv(w)
                if not st["v_done"][i]:
                    st["v"][i]()
                    st["v_done"][i] = True
                    tm["pe"] += 1707

            def drain_qkv_one(wmax):
                for w in range(0, wmax + 1):
                    st = get_qkv(w)
                    for j in range(2 * PAIRS):
                        if not st["qk_done"][j]:
                            ensure_qk(w, j)
                            return True
                    for i in range(4):
                        if not st["v_done"][i]:
                            ensure_v(w, i)
                            return True
                return False

            store = {}
            group_ready = {}
            cproj_pending = []  # (w, [unit closures])

            def run_cproj_unit():
                if not cproj_pending:
                    return False
                w, units = cproj_pending[0]
                units.pop(0)()
                tm["pe"] += 853
                if not units:
                    cproj_pending.pop(0)
                return True

            def force_cproj(w):
                while cproj_pending and cproj_pending[0][0] <= w:
                    run_cproj_unit()

            all_groups = [
                (w, h, g)
                for w in range(NW)
                for h in range(HL)
                for g in range(ngrp(w))
            ]
            prod_pos = 0

            def can_produce():
                if prod_pos >= len(all_groups) or len(store) >= STORE_CAP:
                    return False
                return True

            def produce():
                nonlocal prod_pos, n_prod
                w, h, g = all_groups[prod_pos]
                prod_pos += 1
                pr = h // 2
                diag = 2 * g + 1 >= 4 * w
                ensure_qk(w, pr)
                if diag:
                    ensure_qk(w, PAIRS + pr)
                # time model: scores wait on the s-bank freed by the exp
                # two productions back, then Act runs the exp FIFO.
                scores_dur = sum(
                    (W - max(0, 2 * g + i - 4 * w) * P) * MM
                    for i in range(2)
                )
                lo = max(0, 2 * g - 4 * w) * P
                exp_dur = (2 * W - lo) * 0.833 + 190
                tm["pe"] = max(tm["pe"], sband[n_prod % 2]) + scores_dur
                act_done = max(tm["act"], tm["pe"] + 300) + exp_dur
                tm["act"] = act_done
                sband[n_prod % 2] = act_done
                n_prod += 1
                at = emit_scores_group(w, h, g)
                store[(w, h, g)] = at
                group_ready[(w, h, g)] = act_done + (650 if diag else 250)

            yt_by_w = {}
            for w in range(NW):
                nkb = 4 * w + 4
                yt_w = [
                    yt_pool.tile([P, W], BF16, tag=f"yt{pr}", bufs=2,
                                 name=f"yt{pr}_{w}")
                    for pr in range(PAIRS)
                ]
                yt_by_w[w] = yt_w
                for h in range(HL):
                    pr, sub = h // 2, h % 2
                    if w >= 2 and h == 0:
                        # yt tiles are double-buffered: window w's writes
                        # reuse window w-2's buffers, so c_proj(w-2) must
                        # be fully emitted first.
                        force_cproj(w - 2)
                    # one PSUM bank for the 4 q-block attn@V groups (cols
                    # qb*65..qb*65+64) + spare region for the transposes.
                    y_ps = psum.tile([P, W], F32, tag="y", bufs=2,
                                     name=f"y{w}_{h}")
                    n_av = 0
                    n_av_total = sum(
                        4 - max(0, kb - 4 * w) for kb in range(nkb)
                    )

                    def emit_av_group(g, at):
                        nonlocal n_av
                        for i in range(2):
                            kb = 2 * g + i
                            if kb >= 4 * w:
                                ensure_v(w, kb - 4 * w)
                            for qb in range(max(0, kb - 4 * w), 4):
                                nc.tensor.matmul(
                                    y_ps[:, qb * 65 : qb * 65 + 65],
                                    at[:, i * W + qb * P : i * W + (qb + 1) * P],
                                    v_sb[:, (h * KB + kb) * 65
                                         : (h * KB + kb + 1) * 65],
                                    start=(n_av == 0),
                                    stop=(n_av == n_av_total - 1),
                                    skip_group_check=True,
                                )
                                n_av += 1

                    for g in range(ngrp(w)):
                        key = (w, h, g)
                        while True:
                            # produce-first: keep ~2 exps in flight on Act
                            # (the s-bank double-buffer ceiling) so the Act
                            # engine never starves and the store absorbs
                            # its early idle capacity.
                            if can_produce() and tm["act"] - tm["pe"] < 1800:
                                produce()
                                continue
                            if key in store and group_ready[key] <= tm["pe"]:
                                break
                            if run_cproj_unit():
                                continue
                            if drain_qkv_one(min(w + 1, NW - 1)):
                                continue
                            if can_produce():
                                produce()
                                continue
                            if key in store:
                                tm["pe"] = group_ready[key]
                                break
                            # store full of future groups but key missing
                            # cannot happen (FIFO production); stall-produce
                            produce()
                        at = store.pop(key)
                        av_dur = sum(
                            (4 - max(0, 2 * g + i - 4 * w)) * 65 * MM
                            for i in range(2)
                        )
                        tm["pe"] += av_dur
                        emit_av_group(g, at)

                    # normalize (per-partition scalar) + PE transpose back
                    # to [d, q] through spare regions of the y bank.
                    yn = yn_pool.tile([P, 4 * D], BF16, tag="yn",
                                      name=f"yn{w}_{h}")
                    rc = norm_pool.tile([P, 4], F32, tag="rc", bufs=2,
                                        name=f"rc{w}_{h}")
                    y_qv = y_ps[:, 0 : 4 * 65].rearrange(
                        "p (q c) -> p q c", c=65
                    )
                    nc.vector.reciprocal(rc[:], y_qv[:, :, 64])
                    # the very tail runs on Act (idle once exps are done)
                    # to shorten the last heads' serial normalize chain.
                    on_act = w == NW - 1 and h >= HL - 2
                    for qb in range(4):
                        if on_act:
                            nc.scalar.activation(
                                yn[:, qb * D : (qb + 1) * D],
                                y_ps[:, qb * 65 : qb * 65 + 64],
                                mybir.ActivationFunctionType.Copy,
                                scale=rc[:, qb : qb + 1],
                            )
                        else:
                            nc.vector.tensor_scalar(
                                out=yn[:, qb * D : (qb + 1) * D],
                                in0=y_ps[:, qb * 65 : qb * 65 + 64],
                                scalar1=rc[:, qb : qb + 1],
                                scalar2=None,
                                op0=mybir.AluOpType.mult,
                            )
                    t_ps = [
                        y_ps[0:D, 384:448].bitcast(BF16),
                        y_ps[0:D, 448:512].bitcast(BF16),
                    ]
                    for qb in range(4):
                        tp = t_ps[qb % 2]
                        nc.tensor.transpose(
                            tp, yn[:, qb * D : (qb + 1) * D], ident_t[:]
                        )
                        dst = yt_w[pr][sub * D : (sub + 1) * D,
                                       qb * P : (qb + 1) * P]
                        if on_act:
                            nc.scalar.copy(dst, tp)
                        else:
                            nc.vector.tensor_copy(dst, tp)
                    tm["pe"] += 400

                cproj_pending.append((w, cproj_units(w, yt_w)))

            # drain whatever filler remains
            while run_cproj_unit():
                pass

    nc.compile()
    _CACHE["nc"] = nc
    return nc


def make_in_maps(x, w_attn, b_attn, w_proj, b_proj):
    """Host-side sharding: per-core input dict (matmul operands in bf16)."""
    x = np.ascontiguousarray(np.asarray(x, dtype=np.float32))
    w_attn = np.asarray(w_attn, dtype=np.float32)
    b_attn = np.asarray(b_attn, dtype=np.float32)
    w_proj = np.asarray(w_proj, dtype=np.float32)
    b_proj = np.asarray(b_proj, dtype=np.float32)

    def bf(a):
        return np.ascontiguousarray(a).astype(ml_dtypes.bfloat16)

    trimask = bf(np.triu(np.ones((P, P), np.float32)))  # [k, q]: 1 if q >= k
    ident = bf(np.eye(P, dtype=np.float32))
    in_maps = []
    for c in range(N_CORES):
        b = c // 2
        g = c % 2
        h0 = g * HL
        # Q/K columns arranged pair-wise: [q(h0) q(h0+1) | q(h0+2) ... | k(...)]
        qcols = np.arange(h0 * D, (h0 + HL) * D)
        kcols = C + qcols
        wqk = np.concatenate(
            [w_attn[:, qcols], w_attn[:, kcols]], axis=1
        )  # [C, 1024]
        bqk_flat = np.concatenate([b_attn[qcols], b_attn[kcols]])  # [1024]
        bqk = np.ascontiguousarray(bqk_flat.reshape(2 * PAIRS, P).T)  # [128, 8]
        vcols = 2 * C + np.arange(h0 * D, (h0 + HL) * D)
        wv = np.ascontiguousarray(w_attn[:, vcols])  # [C, 512]
        bv = np.broadcast_to(b_attn[vcols], (P, HL * D)).copy()
        wp = np.ascontiguousarray(w_proj[h0 * D : (h0 + HL) * D, :])  # [512, C]
        if g == 0:
            bpr = np.broadcast_to(b_proj, (P, C)).copy()
        else:
            bpr = np.zeros((P, C), dtype=np.float32)
        in_maps.append(
            {
                "xt": bf(x[b].T),  # [C, T]
                "wqk": bf(wqk),
                "wv": bf(wv),
                "bqk": bqk,
                "bv": bv,
                "wp": bf(wp),
                "bpr": bpr,
                "trimask": trimask,
                "ident": ident,
            }
        )
    return in_maps


def kernel(x, w_attn, b_attn, w_proj, b_proj, _trace=False):
    global LAST_RESULTS
    nc = build_nc()
    in_maps = make_in_maps(x, w_attn, b_attn, w_proj, b_proj)
    res = run_bass_kernel_spmd(
        nc, in_maps, list(range(N_CORES)), trace=_trace
    )
    LAST_RESULTS = res
    outs = [res.results[c]["out"] for c in range(N_CORES)]
    y = np.stack([outs[2 * b] + outs[2 * b + 1] for b in range(B)], axis=0)
    return y.astype(np.float32)
